# revision 10
# baseline (speedup 1.0000x reference)
"""Trainium2 Bass kernel for nn_InundationCoder (2-layer GAT + LSTM + CMAL head).

Key structural insight: the reference only consumes `attention[batch_idx]`
(32 rows), so the whole GAT collapses to the 2-hop in-neighborhood of the 32
seed nodes (~1k active nodes, ~1k active edges, out of 8192/40960).

Host side (inside kernel()): extract the 2-hop subgraph, compact node ids,
sort edges by destination, build one-hot indicator matrices and int16 gather
indices, pack/transpose features, and shard across 8 NeuronCores.

Device side (single SPMD Bass program, 8 cores):
  D1: per-core dense slab  feats -> x -> (h1 | a_s1 | a_d1)     [node-major]
  AG1: AllGather of the combined h1 rows
  E1: per-core edge phase (dst-range sharded): indexed row gather,
      leaky-relu/exp logits, indicator-matmul segment sums, softmax-divide
  D2: per-core x2 -> (h2 | a_s2 | a_d2)  (input is the core's own E1 output)
  AG2: AllGather h2
  E2+tail (replicated on every core): edge phase into the 32 seeds, river
      projection, 32-step LSTM (gate-major, XW preadded via identity matmul),
      CMAL head. Core 0's outputs are returned.
"""

import sys

import numpy as np

sys.path.insert(0, "/opt/trn_rl_repo")

N_CORES = 8
T = 32
H = 64
HEADS = 4
DIM = 16
LSTM_H = 128
KMIX = 3
P = 128

F32 = None  # set after mybir import


def _blockdiag(a):
    # a: [HEADS, DIM] -> [H, HEADS] block diagonal
    m = np.zeros((H, HEADS), np.float32)
    for h in range(HEADS):
        m[h * DIM:(h + 1) * DIM, h] = a[h]
    return m


def _wrap_idx(ids, ec):
    """int16 gather index layout: per 128-chunk, idx i -> (partition i%16,
    col i//16), replicated 8x down the partitions."""
    ids = np.asarray(ids, np.int64)
    assert len(ids) == ec and ec % 128 == 0
    out = np.zeros((16, ec // 16), np.int16)
    for ch in range(ec // 128):
        blk = ids[ch * 128:(ch + 1) * 128].reshape(8, 16).T
        out[:, ch * 8:(ch + 1) * 8] = blk
    return np.tile(out, (8, 1)).copy()


def _edge_side(dst_c, src_c, n_dst, dcount, ec_min=128):
    """Group edges (already valid) by dst block ranges of `dcount`, pad each
    core's list to a common EC. Returns (EC, per-core dict arrays)."""
    n_blocks = n_dst // dcount
    order = np.argsort(dst_c, kind="stable")
    dst_s, src_s = dst_c[order], src_c[order]
    per = []
    for b in range(n_blocks):
        m = (dst_s >= b * dcount) & (dst_s < (b + 1) * dcount)
        per.append((src_s[m], dst_s[m] - b * dcount))
    ec = max(ec_min, max((len(s) for s, _ in per), default=0))
    ec = (ec + 127) // 128 * 128
    cores = []
    for s, d in per:
        ne = len(s)
        sp = np.zeros(ec, np.int64)
        sp[:ne] = s
        ind = np.zeros((ec, dcount), np.float32)
        ind[np.arange(ne), d] = 1.0
        cores.append({
            "sidx": _wrap_idx(sp, ec),
            "ind": ind,
            "indT": np.ascontiguousarray(ind.T),
        })
    return ec, cores


_BUILD_CACHE = {}
DEBUG = False
TRACE = False


def _build_program(dims):
    import concourse.bass as bass
    import concourse.mybir as mybir
    import concourse.tile as tile
    from concourse import bacc

    f32 = mybir.dt.float32
    i16 = mybir.dt.int16
    AF = mybir.ActivationFunctionType
    OP = mybir.AluOpType

    NS0P, NS1P, EC1, EC2, D1c = (
        dims["NS0P"], dims["NS1P"], dims["EC1"], dims["EC2"], dims["D1c"])
    n0c = NS0P // N_CORES          # dense-1 nodes per core
    NT0 = n0c * T                  # dense-1 rowT per core
    assert NT0 % 512 == 0
    ROW = T * 72                   # combined row elements (h|a_s|a_d per t)

    nc = bacc.Bacc(None, target_bir_lowering=False)

    def din(name, shape, dtype=f32):
        return nc.dram_tensor(name, shape, dtype, kind="ExternalInput")

    # ---- external inputs ----
    featsT = din("featsT", [40, NT0])
    Wx = din("Wx", [40, H])
    W1p = din("W1p", [H, 72])
    W2p = din("W2p", [H, 72])
    Wr = din("Wr", [88, H])
    WiP = din("WiP", [H, 4 * LSTM_H])
    WhP = din("WhP", [LSTM_H, 4 * LSTM_H])
    ident = din("ident", [P, P])
    bb = din("bb", [H, 1])
    b1rep = din("b1rep", [P, H])
    b2rep = din("b2rep", [P, H])
    brrep = din("brrep", [P, H])
    bL = din("bL", [P, 4])
    hb = din("hb", [12, 1])
    rcrdT = din("rcrdT", [24, 1024])
    headW = din("headW", [LSTM_H, 4 * KMIX])
    I1 = din("I1", [EC1, D1c])
    I1T = din("I1T", [D1c, EC1])
    sidx1 = din("sidx1", [P, EC1 // 16], i16)
    dd1 = din("dd1", [P, 8], i16)
    I2 = din("I2", [EC2, 32])
    I2T = din("I2T", [32, EC2])
    sidx2 = din("sidx2", [P, EC2 // 16], i16)
    dd2 = din("dd2", [P, 8], i16)

    # ---- outputs ----
    cast_tm = nc.dram_tensor("cast_tm", [T * 32, 4 * KMIX], f32, kind="ExternalOutput")
    if DEBUG:
        dbg_x2 = nc.dram_tensor("dbg_x2", [D1c, T * H], f32, kind="ExternalOutput")
        dbg_samp = nc.dram_tensor("dbg_samp", [32, T * H], f32, kind="ExternalOutput")
        dbg_ser = nc.dram_tensor("dbg_ser", [H, 1024], f32, kind="ExternalOutput")
        dbg_xw = nc.dram_tensor("dbg_xw", [P, T * 128], f32, kind="ExternalOutput")
        dbg_h = nc.dram_tensor("dbg_h", [P, (T + 1) * 32], f32, kind="ExternalOutput")
        dbg_hrow = nc.dram_tensor("dbg_hrow", [P, 1, T * 72], f32, kind="ExternalOutput")
    hT_o = nc.dram_tensor("hT_o", [32, LSTM_H], f32, kind="ExternalOutput")
    cT_o = nc.dram_tensor("cT_o", [32, LSTM_H], f32, kind="ExternalOutput")

    # ---- internal DRAM ----
    h1shard = nc.dram_tensor("h1shard", [NT0, 72], f32)
    h1full = nc.dram_tensor("h1full", [NS0P * T, 72], f32, addr_space="Shared")
    h2shard = nc.dram_tensor("h2shard", [D1c * T, 72], f32)
    h2full = nc.dram_tensor("h2full", [NS1P * T, 72], f32, addr_space="Shared")
    x2loc = nc.dram_tensor("x2loc", [D1c * T, H], f32)
    samp_tmp = nc.dram_tensor("samp_tmp", [32 * T, H], f32)

    def bcast_mid(ap, pos, count):
        return bass.AP(ap.tensor, ap.offset,
                       list(ap.ap[:pos]) + [[0, count]] + list(ap.ap[pos:]))

    with tile.TileContext(nc) as tc:
        with (
            tc.tile_pool(name="cb", bufs=1) as cb,
            tc.tile_pool(name="sb", bufs=3) as sb,
            tc.tile_pool(name="big", bufs=2) as big,
            tc.tile_pool(name="psA", bufs=3, space="PSUM") as psA,
            tc.tile_pool(name="psV", bufs=1, space="PSUM") as psV,
        ):
            # ---- load constants ----
            def ctile(src, shape, dtype=f32):
                t = cb.tile(shape, dtype, tag="c_" + src.name)
                nc.sync.dma_start(t[:], src[:])
                return t

            identT = ctile(ident, [P, P])
            WxT = ctile(Wx, [40, H])
            W1pT = ctile(W1p, [H, 72])
            W2pT = ctile(W2p, [H, 72])
            WrT = ctile(Wr, [88, H])
            WiPT = ctile(WiP, [H, 4 * LSTM_H])
            WhPT = ctile(WhP, [LSTM_H, 4 * LSTM_H])
            bbT = ctile(bb, [H, 1])
            b1T = ctile(b1rep, [P, H])
            b2T = ctile(b2rep, [P, H])
            brT = ctile(brrep, [P, H])
            bLT = ctile(bL, [P, 4])
            hbT = ctile(hb, [12, 1])
            headWT = ctile(headW, [LSTM_H, 4 * KMIX])
            I1Tt = ctile(I1T, [D1c, EC1])
            I2Tt = ctile(I2T, [32, EC2])
            si1 = ctile(sidx1, [P, EC1 // 16], i16)
            dd1t = ctile(dd1, [P, 8], i16)
            si2 = ctile(sidx2, [P, EC2 // 16], i16)
            dd2t = ctile(dd2, [P, 8], i16)

            # =============== D1: dense phase 1 ===============
            sc_d1 = nc.enter_named_scope("D1", False)
            for c in range(NT0 // 512):
                fT = sb.tile([40, 512], f32, tag="fT")
                nc.sync.dma_start(fT[:], featsT[:, c * 512:(c + 1) * 512])
                xps = psA.tile([H, 512], f32, tag="ps_a")
                nc.tensor.matmul(xps[:], WxT[:], fT[:], start=True, stop=True)
                xT = sb.tile([H, 512], f32, tag="xT")
                nc.scalar.activation(xT[:], xps[:], AF.Relu, bias=bbT[:, 0:1])
                hps = psA.tile([72, 512], f32, tag="ps_a")
                nc.tensor.matmul(hps[:], W1pT[:], xT[:], start=True, stop=True)
                h1Tt = sb.tile([72, 512], f32, tag="h1T")
                nc.scalar.copy(h1Tt[:], hps[:])
                for j in range(4):
                    tp = psA.tile([P, 72], f32, tag="ps_a")
                    nc.tensor.transpose(tp[:], h1Tt[:, j * 128:(j + 1) * 128],
                                        identT[0:72, 0:72])
                    hseg = sb.tile([P, 72], f32, tag="hseg")
                    nc.scalar.copy(hseg[:], tp[:])
                    r0 = (c * 4 + j) * 128
                    nc.sync.dma_start(h1shard[r0:r0 + 128, :], hseg[:])

            nc.leave_named_scope("D1", sc_d1[0], False)
            # =============== AG1 ===============
            sc_ag1 = nc.enter_named_scope("AG1", False)
            nc.gpsimd.collective_compute(
                "AllGather", OP.bypass,
                replica_groups=[list(range(N_CORES))],
                ins=[h1shard[:].opt()], outs=[h1full[:].opt()])

            # =============== shared edge-phase builder ===============
            def edge_phase(hfull, Ind, IndT_t, si_t, dd_t, ec, dcount,
                           brow_t, relu, tag):
                """Returns sbuf tile [dcount, T*H] with the GAT layer output
                rows (bias added, optional relu)."""
                hrows = hfull[:].rearrange("(n t) c -> n (t c)", t=T)
                # dedup gather of the dst rows (for a_d)
                ddt = big.tile([P, 1, ROW], f32, tag="G")
                nc.gpsimd.dma_gather(
                    out_ap=ddt[:], in_ap=hrows, idxs_ap=dd_t[:],
                    num_idxs=128, num_idxs_reg=128, elem_size=ROW)
                adB = cb.tile([dcount, 128], f32, tag="adB")
                nc.vector.tensor_copy(
                    adB[:],
                    ddt[0:dcount, 0, :].rearrange("p (t c) -> p t c", c=72)[:, :, 68:72])
                psv = psV.tile([dcount, T * H], f32, tag="psv")
                psd = psV.tile([dcount, T * HEADS], f32, tag="psd")
                nch = ec // 128
                for ch in range(nch):
                    G = big.tile([P, 1, ROW], f32, tag="G")
                    nc.gpsimd.dma_gather(
                        out_ap=G[:], in_ap=hrows,
                        idxs_ap=si_t[:, ch * 8:(ch + 1) * 8],
                        num_idxs=128, num_idxs_reg=128, elem_size=ROW)
                    Gt = G[:, 0, :].rearrange("p (t c) -> p t c", c=72)
                    adps = psA.tile([P, T * HEADS], f32, tag="ps_a")
                    nc.tensor.matmul(adps[:], IndT_t[:, ch * 128:(ch + 1) * 128],
                                     adB[:], start=True, stop=True,
                                     skip_group_check=True)
                    Ee = sb.tile([P, T * HEADS], f32, tag="Ee")
                    nc.vector.tensor_tensor(
                        out=Ee[:].rearrange("p (t h) -> p t h", h=HEADS),
                        in0=Gt[:, :, 64:68],
                        in1=adps[:].rearrange("p (t h) -> p t h", h=HEADS),
                        op=OP.add)
                    lr = sb.tile([P, T * HEADS], f32, tag="lr")
                    nc.scalar.mul(lr[:], Ee[:], 0.2)
                    w = sb.tile([P, T * HEADS], f32, tag="w")
                    nc.vector.tensor_tensor(out=w[:], in0=lr[:], in1=Ee[:], op=OP.max)
                    we = sb.tile([P, T * HEADS], f32, tag="we")
                    nc.scalar.activation(we[:], w[:], AF.Exp)
                    V = big.tile([P, T * H], f32, tag="V")
                    nc.vector.tensor_tensor(
                        out=V[:].rearrange("p (t h d) -> p t h d", h=HEADS, d=DIM),
                        in0=Gt[:, :, 0:64].rearrange("p t (h d) -> p t h d", d=DIM),
                        in1=we[:].rearrange("p (t h) -> p t h", h=HEADS)
                            .to_broadcast([P, T, HEADS, DIM]),
                        op=OP.mult)
                    It = sb.tile([P, dcount], f32, tag="It")
                    nc.sync.dma_start(It[:], Ind[ch * 128:(ch + 1) * 128, :])
                    first, last = ch == 0, ch == nch - 1
                    for q in range(4):
                        nc.tensor.matmul(psv[:, q * 512:(q + 1) * 512], It[:],
                                         V[:, q * 512:(q + 1) * 512],
                                         start=first, stop=last,
                                         skip_group_check=True)
                    nc.tensor.matmul(psd[:], It[:], we[:], start=first, stop=last,
                                     skip_group_check=True)
                rd = cb.tile([dcount, T * HEADS], f32, tag="rd")
                nc.vector.reciprocal(rd[:], psd[:])
                o1 = cb.tile([dcount, T * H], f32, tag="o1")
                nc.vector.tensor_tensor(
                    out=o1[:].rearrange("p (t h d) -> p t h d", h=HEADS, d=DIM),
                    in0=psv[:].rearrange("p (t h d) -> p t h d", h=HEADS, d=DIM),
                    in1=rd[:].rearrange("p (t h) -> p t h", h=HEADS)
                        .to_broadcast([dcount, T, HEADS, DIM]),
                    op=OP.mult)
                o2 = cb.tile([dcount, T * H], f32, tag="o2")
                nc.vector.tensor_tensor(
                    out=o2[:].rearrange("p (t c) -> p t c", c=H),
                    in0=o1[:].rearrange("p (t c) -> p t c", c=H),
                    in1=bcast_mid(brow_t[0:dcount, :], 1, T),
                    op=OP.add)
                if relu:
                    nc.scalar.activation(o2[:], o2[:], AF.Relu)
                return o2

            nc.leave_named_scope("AG1", sc_ag1[0], False)
            sc_e1 = nc.enter_named_scope("E1", False)
            x2t = edge_phase(h1full, I1, I1Tt, si1, dd1t, EC1, D1c,
                             b1T, True, "e1")
            if DEBUG:
                nc.sync.dma_start(dbg_x2[:], x2t[:])
            nc.sync.dma_start(
                x2loc[:].rearrange("(n t) c -> n (t c)", t=T), x2t[:])

            nc.leave_named_scope("E1", sc_e1[0], False)
            # =============== D2: dense phase 2 ===============
            sc_d2 = nc.enter_named_scope("D2", False)
            for c in range(D1c * T // 512):
                x2T = sb.tile([H, 512], f32, tag="x2T")
                for j in range(4):
                    xt2 = sb.tile([P, H], f32, tag="xt2")
                    r0 = (c * 4 + j) * 128
                    nc.sync.dma_start(xt2[:], x2loc[r0:r0 + 128, :])
                    tp2 = psA.tile([H, P], f32, tag="ps_a")
                    nc.tensor.transpose(tp2[:], xt2[:], identT[:])
                    nc.scalar.copy(x2T[:, j * 128:(j + 1) * 128], tp2[:])
                hps2 = psA.tile([72, 512], f32, tag="ps_a")
                nc.tensor.matmul(hps2[:], W2pT[:], x2T[:], start=True, stop=True)
                h2Tt = sb.tile([72, 512], f32, tag="h1T")
                nc.scalar.copy(h2Tt[:], hps2[:])
                for j in range(4):
                    tp = psA.tile([P, 72], f32, tag="ps_a")
                    nc.tensor.transpose(tp[:], h2Tt[:, j * 128:(j + 1) * 128],
                                        identT[0:72, 0:72])
                    hseg = sb.tile([P, 72], f32, tag="hseg")
                    nc.scalar.copy(hseg[:], tp[:])
                    r0 = (c * 4 + j) * 128
                    nc.sync.dma_start(h2shard[r0:r0 + 128, :], hseg[:])

            nc.leave_named_scope("D2", sc_d2[0], False)
            # =============== AG2 ===============
            sc_ag2 = nc.enter_named_scope("AG2", False)
            nc.gpsimd.collective_compute(
                "AllGather", OP.bypass,
                replica_groups=[list(range(N_CORES))],
                ins=[h2shard[:].opt()], outs=[h2full[:].opt()])

            # =============== E2 (replicated) ===============
            nc.leave_named_scope("AG2", sc_ag2[0], False)
            sc_e2 = nc.enter_named_scope("E2", False)
            samp = edge_phase(h2full, I2, I2Tt, si2, dd2t, EC2, 32,
                              b2T, False, "e2")
            if DEBUG:
                nc.sync.dma_start(dbg_samp[:], samp[:])
                dbgG = big.tile([P, 1, T * 72], f32, tag="G")
                nc.gpsimd.dma_gather(
                    out_ap=dbgG[:],
                    in_ap=h2full[:].rearrange("(n t) c -> n (t c)", t=T),
                    idxs_ap=si2[:, 0:8],
                    num_idxs=128, num_idxs_reg=128, elem_size=T * 72)
                nc.sync.dma_start(dbg_hrow[:], dbgG[:])
            nc.sync.dma_start(
                samp_tmp[:].rearrange("(n t) c -> n (t c)", t=T), samp[:])

            nc.leave_named_scope("E2", sc_e2[0], False)
            sc_t1 = nc.enter_named_scope("RIVER", False)
            # =============== tail: river proj ===============
            F2T = cb.tile([88, 1024], f32)
            nc.sync.dma_start(F2T[64:88, :], rcrdT[:])
            for c8 in range(8):
                st = sb.tile([P, H], f32, tag="st")
                nc.sync.dma_start(st[:], samp_tmp[c8 * 128:(c8 + 1) * 128, :])
                tps = psA.tile([H, P], f32, tag="ps_a")
                nc.tensor.transpose(tps[:], st[:], identT[:])
                nc.scalar.copy(F2T[0:64, c8 * 128:(c8 + 1) * 128], tps[:])
            SERT = cb.tile([H, 1024], f32)
            for c8 in range(8):
                psS = psA.tile([P, H], f32, tag="ps_a")
                nc.tensor.matmul(psS[:], F2T[:, c8 * 128:(c8 + 1) * 128], WrT[:],
                                 start=True, stop=True)
                ser = sb.tile([P, H], f32, tag="ser")
                nc.vector.tensor_tensor(out=ser[:], in0=psS[:], in1=brT[:], op=OP.add)
                nc.scalar.activation(ser[:], ser[:], AF.Relu)
                tps2 = psA.tile([H, P], f32, tag="ps_a")
                nc.tensor.transpose(tps2[:], ser[:], identT[:])
                nc.scalar.copy(SERT[:, c8 * 128:(c8 + 1) * 128], tps2[:])

            if DEBUG:
                nc.sync.dma_start(dbg_ser[:], SERT[:])
            nc.leave_named_scope("RIVER", sc_t1[0], False)
            sc_t2 = nc.enter_named_scope("XW", False)
            # =============== tail: XW precompute ===============
            # XW[p, t*128 + k*32 + b] = (series @ WiP + bL)[b, t, gate k, dim p]
            XW = cb.tile([P, T * 128], f32)
            for k in range(4):
                for half in range(2):
                    psXW = psA.tile([P, 512], f32, tag="ps_a")
                    nc.tensor.matmul(psXW[:],
                                     WiPT[:, k * 128:(k + 1) * 128],
                                     SERT[:, half * 512:(half + 1) * 512],
                                     start=True, stop=True)
                    # psXW cols iterate (b', t) with b = half*16 + b'
                    dst = bass.AP(XW[:].tensor, XW[:].offset + k * 32 + half * 16,
                                  [list(XW[:].ap[0]), [1, 16], [128, T]])
                    nc.scalar.activation(
                        dst, psXW[:].rearrange("p (b t) -> p b t", t=T),
                        AF.Identity, bias=bLT[:, k:k + 1])

            nc.leave_named_scope("XW", sc_t2[0], False)
            sc_t3 = nc.enter_named_scope("LSTM", False)
            # =============== tail: LSTM (dbg hook after XW) ===============
            Hall = cb.tile([P, (T + 1) * 32], f32)
            nc.gpsimd.memset(Hall[:, 0:32], 0.0)
            c_prev = None
            for t in range(T):
                psg = psA.tile([P, 128], f32, tag="ps_a")
                for k in range(4):
                    nc.tensor.matmul(psg[:, k * 32:(k + 1) * 32],
                                     WhPT[:, k * 128:(k + 1) * 128],
                                     Hall[:, t * 32:(t + 1) * 32],
                                     start=True, stop=True, skip_group_check=True)
                gs = sb.tile([P, 128], f32, tag="gs")
                nc.vector.tensor_tensor(out=gs[:], in0=psg[:],
                                        in1=XW[:, t * 128:(t + 1) * 128], op=OP.add)
                gsn = sb.tile([P, 128], f32, tag="gsn")
                nc.scalar.activation(gsn[:, 0:96], gs[:, 0:96], AF.Sigmoid)
                nc.scalar.activation(gsn[:, 96:128], gs[:, 96:128], AF.Tanh)
                ig = sb.tile([P, 32], f32, tag="ig")
                nc.vector.tensor_tensor(out=ig[:], in0=gsn[:, 0:32],
                                        in1=gsn[:, 96:128], op=OP.mult)
                c_new = sb.tile([P, 32], f32, tag="c")
                if c_prev is None:
                    nc.vector.tensor_copy(c_new[:], ig[:])
                else:
                    cf = sb.tile([P, 32], f32, tag="cf")
                    nc.vector.tensor_tensor(out=cf[:], in0=c_prev[:],
                                            in1=gsn[:, 32:64], op=OP.mult)
                    nc.vector.tensor_tensor(out=c_new[:], in0=cf[:], in1=ig[:],
                                            op=OP.add)
                tc_t = sb.tile([P, 32], f32, tag="tc")
                nc.scalar.activation(tc_t[:], c_new[:], AF.Tanh)
                nc.vector.tensor_tensor(out=Hall[:, (t + 1) * 32:(t + 2) * 32],
                                        in0=gsn[:, 64:96], in1=tc_t[:], op=OP.mult)
                c_prev = c_new

            if DEBUG:
                nc.sync.dma_start(dbg_xw[:], XW[:])
                nc.sync.dma_start(dbg_h[:], Hall[:])
            psh = psA.tile([32, P], f32, tag="ps_a")
            nc.tensor.transpose(psh[:], Hall[:, T * 32:(T + 1) * 32], identT[:])
            hTo = sb.tile([32, P], f32, tag="hTo")
            nc.scalar.copy(hTo[:], psh[:])
            nc.sync.dma_start(hT_o[:], hTo[:])
            psc = psA.tile([32, P], f32, tag="ps_a")
            nc.tensor.transpose(psc[:], c_prev[:], identT[:])
            cTo = sb.tile([32, P], f32, tag="cTo")
            nc.scalar.copy(cTo[:], psc[:])
            nc.sync.dma_start(cT_o[:], cTo[:])

            nc.leave_named_scope("LSTM", sc_t3[0], False)
            sc_t4 = nc.enter_named_scope("HEAD", False)
            # =============== tail: head + CMAL ===============
            zc = cb.tile([12, 1024], f32)
            for half in range(2):
                psZ = psA.tile([12, 512], f32, tag="ps_a")
                nc.tensor.matmul(psZ[:], headWT[:],
                                 Hall[:, 32 + half * 512: 32 + (half + 1) * 512],
                                 start=True, stop=True)
                nc.scalar.activation(zc[:, half * 512:(half + 1) * 512], psZ[:],
                                     AF.Identity, bias=hbT[:, 0:1])
            # full-tile transforms (partition base 0 only), assembled
            # per-component after the per-chunk transpose via free slices.
            sp = cb.tile([12, 1024], f32, tag="sp")
            nc.scalar.activation(sp[:], zc[:], AF.Exp)
            sp2 = cb.tile([12, 1024], f32, tag="sp2")
            nc.vector.tensor_scalar_add(sp2[:], sp[:], 1.0)
            sp3 = cb.tile([12, 1024], f32, tag="sp3")
            nc.scalar.activation(sp3[:], sp2[:], AF.Ln)
            sp4 = cb.tile([12, 1024], f32, tag="sp4")
            nc.vector.tensor_scalar_add(sp4[:], sp3[:], 1e-3)
            sg = cb.tile([12, 1024], f32, tag="sg")
            nc.scalar.activation(sg[:], zc[:], AF.Sigmoid)
            for c8 in range(8):
                cs = slice(c8 * 128, (c8 + 1) * 128)
                ctz = sb.tile([P, 12], f32, tag="ctz")
                ctp = sb.tile([P, 12], f32, tag="ctp")
                ctg = sb.tile([P, 12], f32, tag="ctg")
                for tl, srct in ((ctz, zc), (ctp, sp4), (ctg, sg)):
                    psC = psA.tile([P, 12], f32, tag="ps_a")
                    nc.tensor.transpose(psC[:], srct[:, cs], identT[0:12, 0:12])
                    nc.scalar.copy(tl[:], psC[:])
                co = sb.tile([P, 12], f32, tag="co")
                nc.vector.tensor_copy(co[:, 0:3], ctz[:, 0:3])
                nc.vector.tensor_copy(co[:, 3:6], ctp[:, 3:6])
                nc.vector.tensor_copy(co[:, 6:9], ctg[:, 6:9])
                cte = sb.tile([P, 3], f32, tag="cte")
                nc.scalar.activation(cte[:], ctz[:, 9:12], AF.Exp)
                s1 = sb.tile([P, 1], f32, tag="s1")
                nc.vector.tensor_reduce(s1[:], cte[:], axis=mybir.AxisListType.X,
                                        op=OP.add)
                r1 = sb.tile([P, 1], f32, tag="r1")
                nc.vector.reciprocal(r1[:], s1[:])
                nc.vector.tensor_tensor(out=co[:, 9:12], in0=cte[:],
                                        in1=r1[:].to_broadcast([P, 3]), op=OP.mult)
                nc.sync.dma_start(cast_tm[cs, :], co[:])

            nc.leave_named_scope("HEAD", sc_t4[0], False)
    nc.finalize()
    return nc


def kernel(**inputs):
    np32 = lambda k: np.asarray(inputs[k], np.float32)
    era5 = np32("era5")
    bc = np32("basinContinuous")
    bd = np32("basinDiscrete")
    rc = np32("riverContinuous")
    rd = np32("riverDiscrete")
    nodes = np.asarray(inputs["nodes"], np.int64)
    ei = np.asarray(inputs["edge_index"], np.int64)
    N, Tt, _ = era5.shape
    B = nodes.shape[0]
    assert Tt == T and B == 32

    # ---- 2-hop extraction ----
    batch_idx = np.concatenate([[0], np.cumsum(nodes)[:-1]]).astype(np.int64)
    src_all = np.concatenate([ei[0], np.arange(N)])
    dst_all = np.concatenate([ei[1], np.arange(N)])

    seed_u = np.unique(batch_idx)
    in_s2 = np.zeros(N, bool)
    in_s2[seed_u] = True
    e2m = in_s2[dst_all]
    s1_extra = np.setdiff1d(np.unique(src_all[e2m]), seed_u)
    S1 = np.concatenate([seed_u, s1_extra])
    # seeds must occupy a known position: compact id of node seed_u[j] is j
    in_s1 = np.zeros(N, bool)
    in_s1[S1] = True
    e1m = in_s1[dst_all]
    s0_extra = np.setdiff1d(np.unique(src_all[e1m]), S1)
    S0 = np.concatenate([S1, s0_extra])
    g2c = np.full(N, -1, np.int64)
    g2c[S0] = np.arange(len(S0))

    NS1P = max(256, -(-len(S1) // 256) * 256)
    D1c = NS1P // N_CORES
    NS0P = max(NS1P, -(-len(S0) // 128) * 128)
    n0c = NS0P // N_CORES

    # edge lists in compact ids
    e1src, e1dst = g2c[src_all[e1m]], g2c[dst_all[e1m]]
    EC1, e1cores = _edge_side(e1dst, e1src, NS1P, D1c)
    # layer-2 edges: dst column = batch element b (handles duplicate seeds)
    e2src_n, e2dst_n = src_all[e2m], dst_all[e2m]
    e2src = g2c[e2src_n]
    EC2 = max(128, -(-len(e2src) // 128) * 128)
    e2s_p = np.zeros(EC2, np.int64)
    e2s_p[:len(e2src)] = e2src
    I2 = np.zeros((EC2, 32), np.float32)
    for bi in range(B):
        I2[np.nonzero(e2dst_n == batch_idx[bi])[0], bi] = 1.0
    dd2_ids = np.zeros(128, np.int64)
    dd2_ids[:B] = g2c[batch_idx]

    # ---- features (node-major, broadcast, transposed) ----
    nreal = len(S0)
    feats = np.zeros((NS0P, T, 40), np.float32)
    feats[:nreal, :, 0:16] = era5[S0]
    feats[:nreal, :, 16:32] = bc[S0][:, None, :]
    feats[:nreal, :, 32:40] = bd[S0][:, None, :]
    featsT = np.ascontiguousarray(feats.reshape(NS0P * T, 40).T)

    # ---- weights ----
    W = {k: np32(k) for k in ["Wc_b", "Wd_b", "b_b", "gat_W1", "gat_asrc1",
         "gat_adst1", "gat_b1", "gat_W2", "gat_asrc2", "gat_adst2", "gat_b2",
         "Wc_r", "Wd_r", "b_r", "lstm_Wi", "lstm_Wh", "lstm_b", "head_W",
         "head_b"]}
    Wx = np.vstack([W["Wc_b"], W["Wd_b"]])
    W1p = np.hstack([W["gat_W1"], W["gat_W1"] @ _blockdiag(W["gat_asrc1"]),
                     W["gat_W1"] @ _blockdiag(W["gat_adst1"])])
    W2p = np.hstack([W["gat_W2"], W["gat_W2"] @ _blockdiag(W["gat_asrc2"]),
                     W["gat_W2"] @ _blockdiag(W["gat_adst2"])])
    Wr = np.vstack([W["Wc_r"], W["Wd_r"]])
    gp = np.concatenate([np.arange(0, 128), np.arange(128, 256),
                         np.arange(384, 512), np.arange(256, 384)])
    WiP = np.ascontiguousarray(W["lstm_Wi"][:, gp])
    WhP = np.ascontiguousarray(W["lstm_Wh"][:, gp])
    bLp = W["lstm_b"][gp]
    rcrdT = np.zeros((24, 1024), np.float32)
    jj = np.arange(1024)
    rcrdT[0:16, :] = rc[jj // 32].T
    rcrdT[16:24, :] = rd[jj // 32].T

    common = {
        "Wx": Wx, "W1p": W1p, "W2p": W2p, "Wr": Wr, "WiP": WiP, "WhP": WhP,
        "ident": np.eye(P, dtype=np.float32),
        "bb": W["b_b"].reshape(H, 1),
        "b1rep": np.tile(W["gat_b1"], (P, 1)),
        "b2rep": np.tile(W["gat_b2"], (P, 1)),
        "brrep": np.tile(W["b_r"], (P, 1)),
        "bL": np.ascontiguousarray(bLp.reshape(4, P).T),
        "hb": W["head_b"].reshape(12, 1),
        "rcrdT": rcrdT,
        "headW": W["head_W"],
        "I2": I2, "I2T": np.ascontiguousarray(I2.T),
        "sidx2": _wrap_idx(e2s_p, EC2), "dd2": _wrap_idx(dd2_ids, 128),
    }
    in_maps = []
    for c in range(N_CORES):
        m = dict(common)
        m["featsT"] = np.ascontiguousarray(
            featsT[:, c * n0c * T:(c + 1) * n0c * T])
        ecd = e1cores[c]
        m["I1"] = ecd["ind"]
        m["I1T"] = ecd["indT"]
        m["sidx1"] = ecd["sidx"]
        m["dd1"] = _wrap_idx(np.arange(c * D1c, (c + 1) * D1c + 0) .tolist()
                             + [0] * (128 - D1c), 128)
        in_maps.append(m)

    dims = (NS0P, NS1P, EC1, EC2, D1c)
    if dims not in _BUILD_CACHE:
        _BUILD_CACHE[dims] = _build_program(
            {"NS0P": NS0P, "NS1P": NS1P, "EC1": EC1, "EC2": EC2, "D1c": D1c})
    nc = _BUILD_CACHE[dims]

    from concourse.bass_utils import run_bass_kernel_spmd
    kernel.last_in_maps = in_maps
    res = run_bass_kernel_spmd(nc, in_maps, core_ids=list(range(N_CORES)))
    kernel.last_result = res
    r0 = res.results[0]
    if DEBUG:
        kernel.dbg = res.results
    cast = r0["cast_tm"].reshape(T, 32, 4 * KMIX).transpose(1, 0, 2).copy()
    return cast, r0["hT_o"].copy(), r0["cT_o"].copy()
